# revision 9
# baseline (speedup 1.0000x reference)
"""Distributed causal self-attention kernel for one TRN2 chip (8 NeuronCores).

Problem: y = CausalSelfAttention(x) with B=2, T=2048, C=1024, 16 heads x 64.

Sharding (per core c = 0..7: heads {2c, 2c+1} of BOTH batches):
  - Q/K/V projections: each core computes its 2 heads' Q,K,V for both
    batches from the full x (both batches resident in SBUF).
  - Attention: fully local; the 4 (batch, head) units per core run the same
    flash-style single pass as a 4-head group.  Scores are kept transposed
    (s^T[k, q]); AV runs "transposed" too: y[q, d] = e^T . [V | 1] so the
    softmax denominator lands as a per-partition column and normalization is
    a cheap per-partition multiply.
  - o_proj: ZERO collectives.  Each core holds a 128-wide slice of the C
    contraction (its 2 heads), so it computes a PARTIAL o = y_c^T . Wo_c for
    the full [T, C] output of both batches; the host sums the 8 bf16 partial
    outputs.  Precision: partials are f32-accumulated in PSUM and rounded
    once to bf16, so the host sum adds only ~1e-3 relative error.
  - y chunks are PE-transposed to y^T[c, q] right after normalization; the
    per-q-tile transposes + partial-o matmuls + output DMAs are deferred one
    tile and woven into the next tile's k-sweep as PE filler.

All matmuls run in bf16 (fp32 accumulation in PSUM); inputs are converted to
bf16 on the host. QK^T matmuls (contraction dim 64) are packed two-per-PE
via tile_position row tiling. Exp is sized to the exact causal area
(diagonal tiles use narrowed strided APs).
"""
import sys
sys.path.insert(0, '/opt/trn_rl_repo')
import numpy as np
import ml_dtypes

B, T, C = 2, 2048, 1024
NH, HD = 16, 64
N_CORES = 8
UPC = 4                   # units per core = 2 heads x 2 batches
SH = UPC * HD             # attention y width per core = 256
NCB = C // 128            # contraction blocks = 8
VH = 68                   # padded [V_h | 1 | pad] group width
VW = UPC * VH             # vhat row width per t-chunk = 272
BF16 = ml_dtypes.bfloat16

_CACHE = {}


def _build(t_len):
    import concourse.bass as bass
    import concourse.bacc as bacc
    import concourse.tile as tile
    import concourse.mybir as mybir
    dt = mybir.dt
    f32, bf16 = dt.float32, dt.bfloat16

    ntc = t_len // 128        # t chunks of 128 (16)
    nqq = t_len // 256        # attention q tiles of 256 (8)
    ntt = t_len // 512        # projection t tiles (4)
    OB = ntc * C              # out cols per batch = 16 * 1024

    nc = bacc.Bacc("TRN2", target_bir_lowering=False, debug=False,
                   num_devices=N_CORES)
    # inputs arrive pre-blocked on the host: [(cblk p) ...] -> [p, cblk*...]
    xT = nc.dram_tensor("xT", [128, 2 * NCB * t_len], bf16, kind="ExternalInput")
    wq = nc.dram_tensor("wqT", [128, NCB * 128], bf16, kind="ExternalInput")
    wk = nc.dram_tensor("wkT", [128, NCB * 128], bf16, kind="ExternalInput")
    wv = nc.dram_tensor("wvT", [128, NCB * 128], bf16, kind="ExternalInput")
    wop = nc.dram_tensor("wopT", [128, C], bf16, kind="ExternalInput")
    tri = nc.dram_tensor("tri", [128, 128], bf16, kind="ExternalInput")
    ident = nc.dram_tensor("ident", [128, 128], bf16, kind="ExternalInput")
    # partial o, [q-within-stripe, (b, stripe, o)] column-major blocks
    out = nc.dram_tensor("out", [128, 2 * OB], bf16, kind="ExternalOutput")

    with tile.TileContext(nc) as tc:
        with tc.tile_pool(name="big", bufs=1) as big, \
             tc.tile_pool(name="epool", bufs=32) as epool, \
             tc.tile_pool(name="ypool", bufs=3) as ypool, \
             tc.tile_pool(name="ytp", bufs=3) as ytp, \
             tc.tile_pool(name="small", bufs=4) as small, \
             tc.tile_pool(name="stp", bufs=4) as stp, \
             tc.tile_pool(name="psqk", bufs=3, space="PSUM") as psqk, \
             tc.tile_pool(name="psaug", bufs=1, space="PSUM") as psaug:

            # ---- resident SBUF tensors ----
            xt = big.tile([128, 2 * NCB * t_len], bf16)   # x^T both b, c-blocked
            wq_sb = big.tile([128, NCB * 128], bf16)
            wk_sb = big.tile([128, NCB * 128], bf16)
            wv_sb = big.tile([128, NCB * 128], bf16)
            wop_sb = big.tile([128, C], bf16)             # Wo^T own 128 c rows
            tri_sb = big.tile([128, 128], bf16)
            id_sb = big.tile([128, 128], bf16)
            qt_sb = big.tile([128, 2 * t_len], bf16)      # Q^T, batch-blocked
            kt_sb = big.tile([128, 2 * t_len], bf16)
            vhat_sb = big.tile([128, ntc * VW], bf16)     # [V_u | 1] per unit

            # Coalesced input DMAs, first-consumer order.  x arrives one
            # (batch, 512-t) tile at a time (all 8 c-blocks, strided).
            def x_tile_dma(b, ti, pairs=False):
                v = xt.rearrange("p (b k t) -> p b k t", b=2, k=NCB)
                s = xT.rearrange("p (b k t) -> p b k t", b=2, k=NCB)
                if pairs:
                    for k in range(0, NCB, 2):
                        nc.sync.dma_start(
                            v[:, b, k:k + 2, ti * 512:(ti + 1) * 512],
                            s[:, b, k:k + 2, ti * 512:(ti + 1) * 512])
                else:
                    nc.sync.dma_start(
                        v[:, b, :, ti * 512:(ti + 1) * 512],
                        s[:, b, :, ti * 512:(ti + 1) * 512])

            nc.sync.dma_start(wq_sb[:, 0:128], wq[:, 0:128])
            x_tile_dma(0, 0, pairs=True)
            nc.sync.dma_start(wq_sb[:, 128:], wq[:, 128:])
            nc.sync.dma_start(wk_sb[:], wk[:])
            nc.sync.dma_start(wv_sb[:], wv[:])
            x_tile_dma(1, 0, pairs=True)
            nc.sync.dma_start(tri_sb[:], tri[:])
            nc.sync.dma_start(id_sb[:], ident[:])
            nc.sync.dma_start(wop_sb[:], wop[:])
            for ti in range(1, ntt):
                x_tile_dma(0, ti)
                x_tile_dma(1, ti)
            nc.gpsimd.memset(vhat_sb[:], 1.0)

            def proj_unit(w_sb, dst, b, t0):
                """One q/k projection psum group for batch b: [d128, t512]."""
                ps = psqk.tile([128, 512], f32, name="ps")
                for k in range(NCB):
                    nc.tensor.matmul(
                        ps[:],
                        lhsT=w_sb[:, k * 128:(k + 1) * 128],
                        rhs=xt[:, (b * NCB + k) * t_len + t0:
                               (b * NCB + k) * t_len + t0 + 512],
                        start=(k == 0), stop=(k == NCB - 1))
                nc.vector.tensor_copy(
                    dst[:, b * t_len + t0: b * t_len + t0 + 512], ps[:])

            def v_unit(b, tch):
                """One V projection psum group: [t128, 2h x 64d] -> vhat."""
                ps = psqk.tile([128, 128], f32, name="ps")
                for k in range(NCB):
                    nc.tensor.matmul(
                        ps[:],
                        lhsT=xt[:, (b * NCB + k) * t_len + tch * 128:
                                (b * NCB + k) * t_len + (tch + 1) * 128],
                        rhs=wv_sb[:, k * 128:(k + 1) * 128],
                        start=(k == 0), stop=(k == NCB - 1))
                nc.vector.tensor_copy(
                    vhat_sb.rearrange("p (c u v) -> p c u v",
                                      c=ntc, u=UPC)[:, tch, 2 * b:2 * b + 2, 0:64],
                    ps.rearrange("p (h d) -> p h d", h=2))

            def proj_units(ti):
                t0 = ti * 512
                us = []
                for b in (0, 1):
                    us.append(lambda b=b: proj_unit(wq_sb, qt_sb, b, t0))
                    us.append(lambda b=b: proj_unit(wk_sb, kt_sb, b, t0))
                    us.extend(lambda b=b, t=tch: v_unit(b, t)
                              for tch in range(ti * 4, ti * 4 + 4))
                return us

            # ---- filler: PE work units woven into ACT-bound QK stretches.
            # Entries are (deadline_qq, fn): the unit must have run before
            # attention_qtile(deadline_qq) starts (projections feeding it).
            fillq = []
            e_pre = {}          # (qq, kb) -> precomputed e tile

            def pump(n=1):
                for _ in range(min(n, len(fillq))):
                    fillq.pop(0)[1]()

            def pump_due(qq):
                while any(d <= qq for d, _ in fillq):
                    pump(1)

            def qk_exp(qq, kb, q0):
                """QK^T + exp for one k-block; returns e tile.  PSUM column
                groups are ordered (h01, b) so each 2KB bank only receives
                matmuls from a single PE row-tile position (bank0 <- rows
                0-63, bank1 <- rows 64-127) -- mixing positions in one bank
                does not compile.  On-diagonal blocks get the causal tri mask
                applied on DVE after the exp."""
                j = kb - 2 * qq
                qa = 128 if j == 1 else 0
                qk = psqk.tile([128, 1024], f32, name="ps")
                for b in (0, 1):
                    for h01 in (0, 1):
                        g = h01 * 2 + b
                        nc.tensor.matmul(
                            qk[:, g * 256 + qa: (g + 1) * 256],
                            lhsT=kt_sb[h01 * 64:(h01 + 1) * 64,
                                       b * t_len + kb * 128:
                                       b * t_len + (kb + 1) * 128],
                            rhs=qt_sb[h01 * 64:(h01 + 1) * 64,
                                      b * t_len + q0 + qa:
                                      b * t_len + q0 + 256],
                            start=True, stop=True,
                            tile_position=(h01 * 64, 0))
                e = epool.tile([128, 1024], bf16, name="e")
                if qa == 0:
                    nc.scalar.activation(e[:], qk[:],
                                         mybir.ActivationFunctionType.Exp,
                                         scale=1.0 / np.sqrt(HD))
                else:
                    e4 = e.rearrange("p (g q) -> p g q", g=UPC)
                    qk4 = qk.rearrange("p (g q) -> p g q", g=UPC)
                    nc.scalar.activation(e4[:, :, qa:256], qk4[:, :, qa:256],
                                         mybir.ActivationFunctionType.Exp,
                                         scale=1.0 / np.sqrt(HD))
                if j >= 0:
                    # causal mask on the (idle) Pool engine: keeps the
                    # exp->mask->AV chain off DVE's in-order queue, which is
                    # busy with psum evacuations.
                    e4 = e.rearrange("p (g q) -> p g q", g=UPC)
                    nc.gpsimd.tensor_mul(
                        e4[:, :, qa:qa + 128], e4[:, :, qa:qa + 128],
                        tri_sb[:].rearrange("p (o q) -> p o q", o=1)
                        .to_broadcast([128, UPC, 128]))
                return e

            def av4(aug, etile, kb, qc, start, stop):
                """AV for one (k-block, q-chunk): per unit, y[q,d]+rowsum.
                qc0 lives at cols u*256+0:65 of aug, qc1 at u*256+68:133 --
                same PSUM bank, one accumulation region per unit: only the
                very first write per bank carries start=True, only the very
                last carries stop=True (pending-zero covers qc1's first
                write)."""
                for u in range(UPC):
                    nc.tensor.matmul(
                        aug[:, u * 256 + qc * VH: u * 256 + qc * VH + VH],
                        lhsT=etile[:, ((u % 2) * 2 + u // 2) * 256 + qc * 128:
                                   ((u % 2) * 2 + u // 2) * 256 + qc * 128 + 128],
                        rhs=vhat_sb[:, kb * VW + u * VH: kb * VW + (u + 1) * VH],
                        start=(start and u % 2 == 0),
                        stop=(stop and u % 2 == 1), skip_group_check=True)

            def norm_mul(aug, qc):
                a4 = aug.rearrange("p (u x) -> p u x", u=UPC)  # x = 256
                recip = small.tile([128, UPC], f32, name="recip")
                nc.vector.reciprocal(
                    recip.rearrange("p (u o) -> p u o", u=UPC),
                    a4[:, :, qc * VH + 64: qc * VH + 65])
                y = ypool.tile([128, SH], bf16, name="y")
                nc.vector.tensor_mul(
                    y.rearrange("p (u d) -> p u d", u=UPC),
                    a4[:, :, qc * VH: qc * VH + 64],
                    recip.rearrange("p (u o) -> p u o", o=1)
                    .to_broadcast([128, UPC, 64]))
                return y

            def transpose_unit(y, yt, qc):
                """y [q128, (b2 h01 d64)] -> yt[:, b, qc, :] = y^T [c, q]."""
                tp = psqk.tile([128, 256], bf16, name="ps")
                for ch in (0, 1):
                    nc.tensor.transpose(
                        tp[:, ch * 128:(ch + 1) * 128],
                        y[:, ch * 128:(ch + 1) * 128], id_sb[:])
                nc.vector.tensor_copy(
                    yt.rearrange("p (c t) -> p c t", c=2)[:, :, qc * 128:
                                                          (qc + 1) * 128],
                    tp.rearrange("p (c t) -> p c t", c=2))

            def partial_o(yt_t, qq, s, b, act=False):
                """Partial o block for (batch b, stripe 2qq+s): [q128, o1024]
                = (own y^T slice)^T @ Wo^T(own c rows); out to DRAM.  The psum
                evacuation is split in halves (less DVE head-of-line blocking)
                and routed to ACT for late tiles where the exp stream is over."""
                ps = psqk.tile([128, 1024], f32, name="ps")
                st = stp.tile([128, C], bf16, name="st")
                for half in (0, 1):
                    nc.tensor.matmul(
                        ps[:, half * 512:(half + 1) * 512],
                        lhsT=yt_t[:, b * 256 + s * 128: b * 256 + s * 128 + 128],
                        rhs=wop_sb[:, half * 512:(half + 1) * 512],
                        start=True, stop=True)
                    if act:
                        nc.scalar.copy(st[:, half * 512:(half + 1) * 512],
                                       ps[:, half * 512:(half + 1) * 512])
                    else:
                        nc.vector.tensor_copy(st[:, half * 512:(half + 1) * 512],
                                              ps[:, half * 512:(half + 1) * 512])
                nc.sync.dma_start(
                    out[:, b * OB + (2 * qq + s) * C: b * OB + (2 * qq + s + 1) * C],
                    st[:])

            def attention_qtile(qq):
                """One 256-wide q tile over all 4 (batch, head) units.  Single
                k-block sweep: QK^T/exp, both q-chunks' AV, and filler units
                interleaved so the PE keeps busy while ACT drains the exps.
                The y normalization runs on DVE right away, but the PE
                transposes + partial-o of this tile are deferred into the
                next tile's sweep (fillq front) to keep them off the PE
                critical path.  The last tile runs its own tail inline."""
                q0 = qq * 256
                nkb = 2 * qq + 2
                last = qq == nqq - 1
                yt = ytp.tile([128, 512], bf16, name="yt")
                aug = psaug.tile([128, 1024], f32, name="aug")
                y0 = None
                for kb in range(nkb):
                    e = e_pre.pop((qq, kb), None)
                    if e is None:
                        e = qk_exp(qq, kb, q0)
                    if kb < nkb - 1:
                        av4(aug, e, kb, 0, start=(kb == 0), stop=False)
                    av4(aug, e, kb, 1, start=False, stop=(kb == nkb - 1))
                    if kb == nkb - 2:
                        y0 = norm_mul(aug, 0)   # qc0 bytes are final now
                        if last:
                            transpose_unit(y0, yt, 0)
                            partial_o(yt, qq, 0, 0, act=True)
                            partial_o(yt, qq, 0, 1, act=True)
                        else:
                            pump(1)
                    elif kb % 2 == 1:
                        pump(1)
                y1 = norm_mul(aug, 1)
                if last:
                    transpose_unit(y1, yt, 1)
                    partial_o(yt, qq, 1, 0, act=True)
                    partial_o(yt, qq, 1, 1, act=True)
                else:
                    act = qq >= nqq - 2   # these fillers run while ACT is idle
                    units = [(lambda a=y0, b=y1, t=yt:
                              (transpose_unit(a, t, 0), transpose_unit(b, t, 1)))]
                    units += [(lambda s=s, b=b, t=yt, q=qq, a=act:
                               partial_o(t, q, s, b, act=a))
                              for s in (0, 1) for b in (0, 1)]
                    for u in reversed(units):
                        fillq.insert(0, (qq + 2, u))

            # ---- schedule ----
            for u in proj_units(0):
                u()
            fillq.extend((2, u) for u in proj_units(1))
            for qq in range(nqq):
                if qq == 2:
                    fillq.extend((4, u) for u in proj_units(2))
                if qq == 4:
                    fillq.extend((6, u) for u in proj_units(3))
                if qq == 5:
                    last = nqq - 1
                    for kb in range(2 * last + 2):
                        fillq.append((7, (lambda k=kb: e_pre.__setitem__(
                            (last, k), qk_exp(last, k, last * 256)))))
                pump_due(qq)
                attention_qtile(qq)
            pump(len(fillq))

    nc.compile()
    return nc


def _tri_np():
    ki = np.arange(128)[:, None]
    qi = np.arange(128)[None, :]
    return (qi >= ki).astype(np.float32).astype(BF16)


def _block(a, w):
    """[C, w] -> [128, NCB*w] partition-blocked bf16."""
    return np.ascontiguousarray(
        a.reshape(NCB, 128, w).transpose(1, 0, 2).reshape(128, NCB * w)).astype(BF16)


def _prep_inputs(x, Wq, Wk, Wv, Wo, t_len):
    tri = _tri_np()
    ident = np.eye(128, dtype=np.float32).astype(BF16)
    xb = np.concatenate([_block(x[0].T, t_len), _block(x[1].T, t_len)], axis=1)
    WoT = np.ascontiguousarray(Wo.T).astype(BF16)
    in_maps = []
    for c in range(N_CORES):
        rs = slice(128 * c, 128 * (c + 1))
        in_maps.append({
            "xT": xb,
            "wqT": _block(Wq[rs, :].T, 128),
            "wkT": _block(Wk[rs, :].T, 128),
            "wvT": _block(Wv[rs, :].T, 128),
            "wopT": WoT[rs, :],
            "tri": tri,
            "ident": ident,
        })
    return in_maps


def _assemble(results, t_len):
    out = np.zeros((B, t_len, C), dtype=np.float32)
    for c in range(N_CORES):
        r = results[c]["out"].astype(np.float32)     # [128, 2*16*1024]
        r = r.reshape(128, B, t_len // 128, C)       # [q-in-stripe, b, stripe, o]
        out += r.transpose(1, 2, 0, 3).reshape(B, t_len, C)
    return out


def get_nc(t_len=T):
    if t_len not in _CACHE:
        _CACHE[t_len] = _build(t_len)
    return _CACHE[t_len]


def kernel(x, Wq, Wk, Wv, Wo):
    from concourse import bass_utils
    x = np.asarray(x, dtype=np.float32)
    nc = get_nc(T)
    in_maps = _prep_inputs(x, np.asarray(Wq), np.asarray(Wk), np.asarray(Wv),
                           np.asarray(Wo), T)
    res = bass_utils.run_bass_kernel_spmd(nc, in_maps, core_ids=list(range(N_CORES)))
    return _assemble(res.results, T)


# revision 15
# speedup vs baseline: 1.0475x; 1.0475x over previous
"""Distributed causal self-attention kernel for one TRN2 chip (8 NeuronCores).

Problem: y = CausalSelfAttention(x) with B=2, T=2048, C=1024, 16 heads x 64.

Sharding (per core c = 0..7: heads {2c, 2c+1} of BOTH batches):
  - Q/K/V projections: each core computes its 2 heads' Q,K,V for both
    batches from the full x (both batches resident in SBUF).
  - Attention: fully local; the 4 (batch, head) units per core run the same
    flash-style single pass as a 4-head group.  Scores are kept transposed
    (s^T[k, q]); AV runs "transposed" too: y[q, d] = e^T . [V | 1] so the
    softmax denominator lands as a per-partition column and normalization is
    a cheap per-partition multiply.
  - o_proj: ZERO collectives.  Each core holds a 128-wide slice of the C
    contraction (its 2 heads), so it computes a PARTIAL o = y_c^T . Wo_c for
    the full [T, C] output of both batches; the host sums the 8 bf16 partial
    outputs.  Precision: partials are f32-accumulated in PSUM and rounded
    once to bf16, so the host sum adds only ~1e-3 relative error.
  - y chunks are PE-transposed to y^T[c, q] right after normalization; the
    per-q-tile transposes + partial-o matmuls + output DMAs are deferred one
    tile and woven into the next tile's k-sweep as PE filler.

All matmuls run in bf16 (fp32 accumulation in PSUM); inputs are converted to
bf16 on the host. QK^T matmuls (contraction dim 64) are packed two-per-PE
via tile_position row tiling. Exp is sized to the exact causal area
(diagonal tiles use narrowed strided APs).
"""
import sys
sys.path.insert(0, '/opt/trn_rl_repo')
import numpy as np
import ml_dtypes

B, T, C = 2, 2048, 1024
NH, HD = 16, 64
N_CORES = 8
UPC = 4                   # units per core = 2 heads x 2 batches
SH = UPC * HD             # attention y width per core = 256
NCB = C // 128            # contraction blocks = 8
VH = 68                   # padded [V_h | 1 | pad] group width
VW = UPC * VH             # vhat row width per t-chunk = 272
BF16 = ml_dtypes.bfloat16

_CACHE = {}


def _build(t_len):
    import concourse.bass as bass
    import concourse.bacc as bacc
    import concourse.tile as tile
    import concourse.mybir as mybir
    dt = mybir.dt
    f32, bf16 = dt.float32, dt.bfloat16

    ntc = t_len // 128        # t chunks of 128 (16)
    nqq = t_len // 256        # attention q tiles of 256 (8)
    ntt = t_len // 512        # projection t tiles (4)
    OB = ntc * C              # out cols per batch = 16 * 1024

    nc = bacc.Bacc("TRN2", target_bir_lowering=False, debug=False,
                   num_devices=N_CORES)
    # inputs arrive pre-blocked on the host: [(cblk p) ...] -> [p, cblk*...]
    xT = nc.dram_tensor("xT", [128, 2 * NCB * t_len], bf16, kind="ExternalInput")
    wq = nc.dram_tensor("wqT", [128, NCB * 128], bf16, kind="ExternalInput")
    wk = nc.dram_tensor("wkT", [128, NCB * 128], bf16, kind="ExternalInput")
    wv = nc.dram_tensor("wvT", [128, NCB * 128], bf16, kind="ExternalInput")
    wop = nc.dram_tensor("wopT", [128, C], bf16, kind="ExternalInput")
    tri = nc.dram_tensor("tri", [128, 128], bf16, kind="ExternalInput")
    ident = nc.dram_tensor("ident", [128, 128], bf16, kind="ExternalInput")
    # partial o, [q-within-stripe, (b, stripe, o)] column-major blocks
    out = nc.dram_tensor("out", [128, 2 * OB], bf16, kind="ExternalOutput")

    with tile.TileContext(nc) as tc:
        with tc.tile_pool(name="big", bufs=1) as big, \
             tc.tile_pool(name="epool", bufs=32) as epool, \
             tc.tile_pool(name="ypool", bufs=3) as ypool, \
             tc.tile_pool(name="ytp", bufs=3) as ytp, \
             tc.tile_pool(name="small", bufs=4) as small, \
             tc.tile_pool(name="stp", bufs=4) as stp, \
             tc.tile_pool(name="psqk", bufs=3, space="PSUM") as psqk, \
             tc.tile_pool(name="psaug", bufs=1, space="PSUM") as psaug:

            # ---- resident SBUF tensors ----
            xt = big.tile([128, 2 * NCB * t_len], bf16)   # x^T both b, c-blocked
            wq_sb = big.tile([128, NCB * 128], bf16)
            wk_sb = big.tile([128, NCB * 128], bf16)
            wv_sb = big.tile([128, NCB * 128], bf16)
            wop_sb = big.tile([128, C], bf16)             # Wo^T own 128 c rows
            tri_sb = big.tile([128, 128], bf16)
            id_sb = big.tile([128, 128], bf16)
            qt_sb = big.tile([128, 2 * t_len], bf16)      # Q^T, batch-blocked
            kt_sb = big.tile([128, 2 * t_len], bf16)
            vhat_sb = big.tile([128, ntc * VW], bf16)     # [V_u | 1] per unit

            # Coalesced input DMAs, first-consumer order.  x arrives one
            # (batch, 512-t) tile at a time (all 8 c-blocks, strided).
            def x_tile_dma(b, ti, pairs=False):
                v = xt.rearrange("p (b k t) -> p b k t", b=2, k=NCB)
                s = xT.rearrange("p (b k t) -> p b k t", b=2, k=NCB)
                if pairs:
                    for k in range(0, NCB, 2):
                        nc.sync.dma_start(
                            v[:, b, k:k + 2, ti * 512:(ti + 1) * 512],
                            s[:, b, k:k + 2, ti * 512:(ti + 1) * 512])
                else:
                    nc.sync.dma_start(
                        v[:, b, :, ti * 512:(ti + 1) * 512],
                        s[:, b, :, ti * 512:(ti + 1) * 512])

            nc.sync.dma_start(wq_sb[:, 0:128], wq[:, 0:128])
            x_tile_dma(0, 0, pairs=True)
            nc.sync.dma_start(wq_sb[:, 128:], wq[:, 128:])
            nc.sync.dma_start(wk_sb[:], wk[:])
            nc.sync.dma_start(wv_sb[:], wv[:])
            x_tile_dma(1, 0, pairs=True)
            nc.sync.dma_start(tri_sb[:], tri[:])
            nc.sync.dma_start(id_sb[:], ident[:])
            nc.sync.dma_start(wop_sb[:], wop[:])
            for ti in range(1, ntt):
                x_tile_dma(0, ti)
                x_tile_dma(1, ti)
            nc.gpsimd.memset(vhat_sb[:], 1.0)

            def proj_unit(w_sb, dst, b, t0):
                """One q/k projection psum group for batch b: [d128, t512]."""
                ps = psqk.tile([128, 512], f32, name="ps")
                for k in range(NCB):
                    nc.tensor.matmul(
                        ps[:],
                        lhsT=w_sb[:, k * 128:(k + 1) * 128],
                        rhs=xt[:, (b * NCB + k) * t_len + t0:
                               (b * NCB + k) * t_len + t0 + 512],
                        start=(k == 0), stop=(k == NCB - 1))
                nc.vector.tensor_copy(
                    dst[:, b * t_len + t0: b * t_len + t0 + 512], ps[:])

            def v_unit(b, tch):
                """One V projection psum group: [t128, 2h x 64d] -> vhat."""
                ps = psqk.tile([128, 128], f32, name="ps")
                for k in range(NCB):
                    nc.tensor.matmul(
                        ps[:],
                        lhsT=xt[:, (b * NCB + k) * t_len + tch * 128:
                                (b * NCB + k) * t_len + (tch + 1) * 128],
                        rhs=wv_sb[:, k * 128:(k + 1) * 128],
                        start=(k == 0), stop=(k == NCB - 1))
                nc.vector.tensor_copy(
                    vhat_sb.rearrange("p (c u v) -> p c u v",
                                      c=ntc, u=UPC)[:, tch, 2 * b:2 * b + 2, 0:64],
                    ps.rearrange("p (h d) -> p h d", h=2))

            def proj_units(ti):
                t0 = ti * 512
                us = []
                for b in (0, 1):
                    us.append(lambda b=b: proj_unit(wq_sb, qt_sb, b, t0))
                    us.append(lambda b=b: proj_unit(wk_sb, kt_sb, b, t0))
                    us.extend(lambda b=b, t=tch: v_unit(b, t)
                              for tch in range(ti * 4, ti * 4 + 4))
                return us

            # ---- filler: PE work units woven into ACT-bound QK stretches.
            # Entries are (deadline_qq, fn): the unit must have run before
            # attention_qtile(deadline_qq) starts (projections feeding it).
            fillq = []
            e_pre = {}          # (qq, kb) -> precomputed e tile

            def pump(n=1):
                for _ in range(min(n, len(fillq))):
                    fillq.pop(0)[1]()

            def pump_due(qq):
                while any(d <= qq for d, _ in fillq):
                    pump(1)

            def qk_exp(qq, kb, q0):
                """QK^T + exp for one k-block; returns e tile.  PSUM column
                groups are ordered (h01, b) so each 2KB bank only receives
                matmuls from a single PE row-tile position (bank0 <- rows
                0-63, bank1 <- rows 64-127) -- mixing positions in one bank
                does not compile.  On-diagonal blocks get the causal tri mask
                applied on DVE after the exp."""
                j = kb - 2 * qq
                qa = 128 if j == 1 else 0
                qk = psqk.tile([128, 1024], f32, name="ps")
                for b in (0, 1):
                    for h01 in (0, 1):
                        g = h01 * 2 + b
                        nc.tensor.matmul(
                            qk[:, g * 256 + qa: (g + 1) * 256],
                            lhsT=kt_sb[h01 * 64:(h01 + 1) * 64,
                                       b * t_len + kb * 128:
                                       b * t_len + (kb + 1) * 128],
                            rhs=qt_sb[h01 * 64:(h01 + 1) * 64,
                                      b * t_len + q0 + qa:
                                      b * t_len + q0 + 256],
                            start=True, stop=True,
                            tile_position=(h01 * 64, 0))
                e = epool.tile([128, 1024], bf16, name="e")
                if qa == 0:
                    nc.scalar.activation(e[:], qk[:],
                                         mybir.ActivationFunctionType.Exp,
                                         scale=1.0 / np.sqrt(HD))
                else:
                    e4 = e.rearrange("p (g q) -> p g q", g=UPC)
                    qk4 = qk.rearrange("p (g q) -> p g q", g=UPC)
                    nc.scalar.activation(e4[:, :, qa:256], qk4[:, :, qa:256],
                                         mybir.ActivationFunctionType.Exp,
                                         scale=1.0 / np.sqrt(HD))
                if j >= 0:
                    e4 = e.rearrange("p (g q) -> p g q", g=UPC)
                    nc.vector.tensor_mul(
                        e4[:, :, qa:qa + 128], e4[:, :, qa:qa + 128],
                        tri_sb[:].rearrange("p (o q) -> p o q", o=1)
                        .to_broadcast([128, UPC, 128]))
                return e

            def av4(aug, etile, kb, qc, start, stop):
                """AV for one (k-block, q-chunk): per unit, y[q,d]+rowsum.
                qc0 lives at cols u*256+0:65 of aug, qc1 at u*256+68:133 --
                same PSUM bank, one accumulation region per unit: only the
                very first write per bank carries start=True, only the very
                last carries stop=True (pending-zero covers qc1's first
                write)."""
                for u in range(UPC):
                    nc.tensor.matmul(
                        aug[:, u * 256 + qc * VH: u * 256 + qc * VH + VH],
                        lhsT=etile[:, ((u % 2) * 2 + u // 2) * 256 + qc * 128:
                                   ((u % 2) * 2 + u // 2) * 256 + qc * 128 + 128],
                        rhs=vhat_sb[:, kb * VW + u * VH: kb * VW + (u + 1) * VH],
                        start=(start and u % 2 == 0),
                        stop=(stop and u % 2 == 1), skip_group_check=True)

            def norm_mul(aug, qc):
                a4 = aug.rearrange("p (u x) -> p u x", u=UPC)  # x = 256
                recip = small.tile([128, UPC], f32, name="recip")
                nc.vector.reciprocal(
                    recip.rearrange("p (u o) -> p u o", u=UPC),
                    a4[:, :, qc * VH + 64: qc * VH + 65])
                y = ypool.tile([128, SH], bf16, name="y")
                nc.vector.tensor_mul(
                    y.rearrange("p (u d) -> p u d", u=UPC),
                    a4[:, :, qc * VH: qc * VH + 64],
                    recip.rearrange("p (u o) -> p u o", o=1)
                    .to_broadcast([128, UPC, 64]))
                return y

            def transpose_unit(y, yt, qc):
                """y [q128, (b2 h01 d64)] -> yt[:, b, qc, :] = y^T [c, q]."""
                tp = psqk.tile([128, 256], bf16, name="ps")
                for ch in (0, 1):
                    nc.tensor.transpose(
                        tp[:, ch * 128:(ch + 1) * 128],
                        y[:, ch * 128:(ch + 1) * 128], id_sb[:])
                nc.vector.tensor_copy(
                    yt.rearrange("p (c t) -> p c t", c=2)[:, :, qc * 128:
                                                          (qc + 1) * 128],
                    tp.rearrange("p (c t) -> p c t", c=2))

            def partial_o(yt_t, qq, s, b, act=False):
                """Partial o block for (batch b, stripe 2qq+s): [q128, o1024]
                = (own y^T slice)^T @ Wo^T(own c rows); out to DRAM.  The psum
                evacuation is split in halves (less DVE head-of-line blocking)
                and routed to ACT for late tiles where the exp stream is over."""
                ps = psqk.tile([128, 1024], f32, name="ps")
                st = stp.tile([128, C], bf16, name="st")
                for half in (0, 1):
                    nc.tensor.matmul(
                        ps[:, half * 512:(half + 1) * 512],
                        lhsT=yt_t[:, b * 256 + s * 128: b * 256 + s * 128 + 128],
                        rhs=wop_sb[:, half * 512:(half + 1) * 512],
                        start=True, stop=True)
                    if act:
                        nc.scalar.copy(st[:, half * 512:(half + 1) * 512],
                                       ps[:, half * 512:(half + 1) * 512])
                    else:
                        nc.vector.tensor_copy(st[:, half * 512:(half + 1) * 512],
                                              ps[:, half * 512:(half + 1) * 512])
                nc.sync.dma_start(
                    out[:, b * OB + (2 * qq + s) * C: b * OB + (2 * qq + s + 1) * C],
                    st[:])

            def attention_qtile(qq):
                """One 256-wide q tile over all 4 (batch, head) units.  Single
                k-block sweep: QK^T/exp, both q-chunks' AV, and filler units
                interleaved so the PE keeps busy while ACT drains the exps.
                The y normalization runs on DVE right away, but the PE
                transposes + partial-o of this tile are deferred into the
                next tile's sweep (fillq front) to keep them off the PE
                critical path.  The last tile runs its own tail inline."""
                q0 = qq * 256
                nkb = 2 * qq + 2
                last = qq == nqq - 1
                yt = ytp.tile([128, 512], bf16, name="yt")
                aug = psaug.tile([128, 1024], f32, name="aug")
                y0 = None

                def make_e(kb):
                    e = e_pre.pop((qq, kb), None)
                    return e if e is not None else qk_exp(qq, kb, q0)

                # one-block lookahead: block kb+1's QK matmuls are emitted
                # before block kb's AV so the exp+mask latency of kb+1 is
                # covered by PE work.
                e_next = make_e(0)
                for kb in range(nkb):
                    e = e_next
                    if kb + 1 < nkb:
                        e_next = make_e(kb + 1)
                    else:
                        pump(1)
                    if kb < nkb - 1:
                        av4(aug, e, kb, 0, start=(kb == 0), stop=False)
                    av4(aug, e, kb, 1, start=False, stop=(kb == nkb - 1))
                    if kb == nkb - 2:
                        y0 = norm_mul(aug, 0)   # qc0 bytes are final now
                        if last:
                            transpose_unit(y0, yt, 0)
                            partial_o(yt, qq, 0, 0, act=True)
                            partial_o(yt, qq, 0, 1, act=True)
                        else:
                            pump(1)
                    elif kb % 2 == 1:
                        pump(1)
                y1 = norm_mul(aug, 1)
                if last:
                    transpose_unit(y1, yt, 1)
                    partial_o(yt, qq, 1, 0, act=True)
                    partial_o(yt, qq, 1, 1, act=True)
                else:
                    act = qq >= nqq - 2   # these fillers run while ACT is idle
                    units = [(lambda a=y0, b=y1, t=yt:
                              (transpose_unit(a, t, 0), transpose_unit(b, t, 1)))]
                    units += [(lambda s=s, b=b, t=yt, q=qq, a=act:
                               partial_o(t, q, s, b, act=a))
                              for s in (0, 1) for b in (0, 1)]
                    for u in reversed(units):
                        fillq.insert(0, (qq + 2, u))

            # ---- schedule ----
            for u in proj_units(0):
                u()
            fillq.extend((2, u) for u in proj_units(1))
            for qq in range(nqq):
                if qq == 2:
                    fillq.extend((4, u) for u in proj_units(2))
                if qq == 4:
                    fillq.extend((6, u) for u in proj_units(3))
                if qq == 5:
                    last = nqq - 1
                    for kb in range(2 * last + 2):
                        fillq.append((7, (lambda k=kb: e_pre.__setitem__(
                            (last, k), qk_exp(last, k, last * 256)))))
                pump_due(qq)
                attention_qtile(qq)
            pump(len(fillq))

    nc.compile()
    return nc


def _tri_np():
    ki = np.arange(128)[:, None]
    qi = np.arange(128)[None, :]
    return (qi >= ki).astype(np.float32).astype(BF16)


def _block(a, w):
    """[C, w] -> [128, NCB*w] partition-blocked bf16."""
    return np.ascontiguousarray(
        a.reshape(NCB, 128, w).transpose(1, 0, 2).reshape(128, NCB * w)).astype(BF16)


def _prep_inputs(x, Wq, Wk, Wv, Wo, t_len):
    tri = _tri_np()
    ident = np.eye(128, dtype=np.float32).astype(BF16)
    xb = np.concatenate([_block(x[0].T, t_len), _block(x[1].T, t_len)], axis=1)
    WoT = np.ascontiguousarray(Wo.T).astype(BF16)
    in_maps = []
    for c in range(N_CORES):
        rs = slice(128 * c, 128 * (c + 1))
        in_maps.append({
            "xT": xb,
            "wqT": _block(Wq[rs, :].T, 128),
            "wkT": _block(Wk[rs, :].T, 128),
            "wvT": _block(Wv[rs, :].T, 128),
            "wopT": WoT[rs, :],
            "tri": tri,
            "ident": ident,
        })
    return in_maps


def _assemble(results, t_len):
    out = np.zeros((B, t_len, C), dtype=np.float32)
    for c in range(N_CORES):
        r = results[c]["out"].astype(np.float32)     # [128, 2*16*1024]
        r = r.reshape(128, B, t_len // 128, C)       # [q-in-stripe, b, stripe, o]
        out += r.transpose(1, 2, 0, 3).reshape(B, t_len, C)
    return out


def get_nc(t_len=T):
    if t_len not in _CACHE:
        _CACHE[t_len] = _build(t_len)
    return _CACHE[t_len]


def kernel(x, Wq, Wk, Wv, Wo):
    from concourse import bass_utils
    x = np.asarray(x, dtype=np.float32)
    nc = get_nc(T)
    in_maps = _prep_inputs(x, np.asarray(Wq), np.asarray(Wk), np.asarray(Wv),
                           np.asarray(Wo), T)
    res = bass_utils.run_bass_kernel_spmd(nc, in_maps, core_ids=list(range(N_CORES)))
    return _assemble(res.results, T)
